# revision 2
# baseline (speedup 1.0000x reference)
"""GroupQueryAttention (B=2,T=S=2048,E=1024,H=16,HD=64) on 8 trn2 NeuronCores.

Sharding: 32 (batch, head) instances -> 8 cores; core c handles batch c//4,
heads 4*(c%4) .. 4*(c%4)+3 (tensor-parallel on heads + data-parallel on batch).

Host<->device traffic is the wall-clock bottleneck (axon tunnel ~40-80 MB/s),
so inputs are shipped fully deduplicated and reassembled on-chip:
  - core c uploads only T-quarter c%4 of its batch's qT/kT [E,512] bf16;
    an on-device AllGather over {4b..4b+3} rebuilds the full [E,2048].
  - each (batch, head-group) core uploads a distinct 128-column slice of
    Wq/Wkv (and 128 rows of Wo); an AllGather over {c, c+4} rebuilds the
    256-wide head-group slice.
  - the 4 per-core output partials y_c [T,E] f32 are summed on-device with
    a ReduceScatter; each core emits only its [512,1024] slice, cast bf16.

Per-core pipeline (all matmuls bf16 operands, fp32 PSUM accumulation):
  qT = (Wq_c * 1/sqrt(HD))^T-free proj      [256, T]   (lhsT=Wq slice, rhs=query^T)
  kT = Wkv_k_c proj                          [256, S]
  v  = Wkv_v_c proj -> [S, 4*65] with a ones column per head (softmax-sum trick)
  per head pair (row-tiled 64x128 PE mode, T0/T8 concurrent):
    scoresT[s,t] = kT_h^T-slice x qT_h      exp() on ACT -> expT (bf16)
    AV: outT_unnorm[65, t] += v_aug^T-slice x expT   (split K=64 accumulators)
  normalize rows by row 64 (the exp sums), -> outT [256, T]
  y_partial = outT^T x Wo_c  [T, E] f32 -> ReduceScatter -> y [512, E] bf16.
"""

import sys

sys.path.insert(0, "/opt/trn_rl_repo")

from contextlib import ExitStack

import numpy as np
import ml_dtypes

import concourse.bass as bass
import concourse.bacc as bacc
import concourse.tile as tile
from concourse import mybir
from concourse.bass_utils import run_bass_kernel_spmd

B, T, S, E = 2, 2048, 2048, 1024
H, HD = 16, 64
P = 128
TQ = T // 4       # per-core uploaded T/S quarter
NT = 512          # matmul free-dim tile
KCH = E // P      # 8 contraction chunks for projections
SCH = S // P      # 16 key chunks
TCH = T // P      # 16 query chunks
HPC = 4           # heads per core
SCALE = 1.0 / np.sqrt(HD)

F32 = mybir.dt.float32
BF16 = mybir.dt.bfloat16
EXPF = mybir.ActivationFunctionType.Exp

BATCH_GROUPS = [[0, 1, 2, 3], [4, 5, 6, 7]]
PAIR_GROUPS = [[0, 4], [1, 5], [2, 6], [3, 7]]

_prog_cache = {}


def _build_program():
    if "nc" in _prog_cache:
        return _prog_cache["nc"]

    nc = bacc.Bacc("TRN2", target_bir_lowering=False, debug=False, num_devices=8)

    qp_d = nc.dram_tensor("qp", [E, TQ], BF16, kind="ExternalInput").ap()
    kp_d = nc.dram_tensor("kp", [E, TQ], BF16, kind="ExternalInput").ap()
    wq_d = nc.dram_tensor("wq", [E, P], BF16, kind="ExternalInput").ap()
    wkk_d = nc.dram_tensor("wkk", [E, P], BF16, kind="ExternalInput").ap()
    wkv_d = nc.dram_tensor("wkv", [E, P], BF16, kind="ExternalInput").ap()
    wo_d = nc.dram_tensor("wo", [P, E], BF16, kind="ExternalInput").ap()
    y_d = nc.dram_tensor("y", [TQ, E], BF16, kind="ExternalOutput").ap()

    # DRAM bounce buffers (collectives can't touch I/O tensors)
    qp_b = nc.dram_tensor("qp_b", [E, TQ], BF16)
    kp_b = nc.dram_tensor("kp_b", [E, TQ], BF16)
    wq_b = nc.dram_tensor("wq_b", [E, P], BF16)
    wkk_b = nc.dram_tensor("wkk_b", [E, P], BF16)
    wkv_b = nc.dram_tensor("wkv_b", [E, P], BF16)
    wo_b = nc.dram_tensor("wo_b", [P, E], BF16)
    qg = nc.dram_tensor("qg", [4 * E, TQ], BF16)     # chunk i = T-quarter i
    kg = nc.dram_tensor("kg", [4 * E, TQ], BF16)
    wqg = nc.dram_tensor("wqg", [2 * E, P], BF16)    # chunk j = col half j
    wkkg = nc.dram_tensor("wkkg", [2 * E, P], BF16)
    wkvg = nc.dram_tensor("wkvg", [2 * E, P], BF16)
    wog = nc.dram_tensor("wog", [2 * P, E], BF16)    # chunk j = row half j
    yp = nc.dram_tensor("yp", [T, E], F32)           # per-core partial
    yr = nc.dram_tensor("yr", [TQ, E], F32)          # reduce-scattered slice

    with tile.TileContext(nc) as tc, ExitStack() as ctx:
        const = ctx.enter_context(tc.tile_pool(name="const", bufs=1))

        # ---- gather sharded inputs on-chip ---------------------------------
        nc.gpsimd.dma_start(qp_b.ap(), qp_d)
        nc.gpsimd.dma_start(kp_b.ap(), kp_d)
        nc.gpsimd.dma_start(wq_b.ap(), wq_d)
        nc.gpsimd.dma_start(wkk_b.ap(), wkk_d)
        nc.gpsimd.dma_start(wkv_b.ap(), wkv_d)
        nc.gpsimd.dma_start(wo_b.ap(), wo_d)
        for src, dst, groups in (
            (qp_b, qg, BATCH_GROUPS),
            (kp_b, kg, BATCH_GROUPS),
            (wq_b, wqg, PAIR_GROUPS),
            (wkk_b, wkkg, PAIR_GROUPS),
            (wkv_b, wkvg, PAIR_GROUPS),
            (wo_b, wog, PAIR_GROUPS),
        ):
            nc.gpsimd.collective_compute(
                "AllGather",
                mybir.AluOpType.bypass,
                replica_groups=groups,
                ins=[src.ap().opt()],
                outs=[dst.ap().opt()],
            )

        # ---- resident loads -------------------------------------------------
        qTc = []
        kTc = []
        wq = []
        wkk = []
        wkv = []
        for k in range(KCH):
            t_q = const.tile([P, T], BF16, tag=f"qTc{k}", name=f"qTc{k}")
            t_k = const.tile([P, S], BF16, tag=f"kTc{k}", name=f"kTc{k}")
            for i in range(4):
                nc.sync.dma_start(
                    t_q[:, i * TQ : (i + 1) * TQ],
                    qg.ap()[i * E + k * P : i * E + (k + 1) * P, :],
                )
                nc.sync.dma_start(
                    t_k[:, i * TQ : (i + 1) * TQ],
                    kg.ap()[i * E + k * P : i * E + (k + 1) * P, :],
                )
            qTc.append(t_q)
            kTc.append(t_k)
            t = const.tile([P, HPC * HD], BF16, tag=f"wq{k}", name=f"wq{k}")
            for j in range(2):
                nc.sync.dma_start(
                    t[:, j * P : (j + 1) * P],
                    wqg.ap()[j * E + k * P : j * E + (k + 1) * P, :],
                )
            wq.append(t)
            t = const.tile([P, HPC * HD], BF16, tag=f"wkk{k}", name=f"wkk{k}")
            for j in range(2):
                nc.sync.dma_start(
                    t[:, j * P : (j + 1) * P],
                    wkkg.ap()[j * E + k * P : j * E + (k + 1) * P, :],
                )
            wkk.append(t)
            t = const.tile([P, HPC * HD], BF16, tag=f"wkv{k}", name=f"wkv{k}")
            for j in range(2):
                nc.sync.dma_start(
                    t[:, j * P : (j + 1) * P],
                    wkvg.ap()[j * E + k * P : j * E + (k + 1) * P, :],
                )
            wkv.append(t)
        wo = []
        for k in range(2):
            t = const.tile([P, E], BF16, tag=f"wo{k}", name=f"wo{k}")
            nc.sync.dma_start(t[:], wog.ap()[k * P : (k + 1) * P, :])
            wo.append(t)

        # persistent intermediates
        qt_sb = [const.tile([P, T], BF16, tag=f"qt{m}", name=f"qt{m}") for m in range(2)]
        kt_sb = [const.tile([P, S], BF16, tag=f"kt{m}", name=f"kt{m}") for m in range(2)]
        v_sb = [const.tile([P, HPC * (HD + 1)], BF16, tag=f"v{s}", name=f"v{s}") for s in range(SCH)]
        outt_sb = [const.tile([P, T], BF16, tag=f"ot{m}", name=f"ot{m}") for m in range(2)]

        # ---- projections ----------------------------------------------------
        with tc.tile_pool(name="pp_proj", bufs=2, space="PSUM") as pp:
            # qT / kT projections: out [128(m), 512(n)] over K=E
            for dst, w, src in ((qt_sb, wq, qTc), (kt_sb, wkk, kTc)):
                for m in range(2):
                    for n in range(T // NT):
                        ps = pp.tile([P, NT], F32, tag="proj", name="proj")
                        for k in range(KCH):
                            nc.tensor.matmul(
                                ps[:],
                                w[k][:, m * P : (m + 1) * P],
                                src[k][:, n * NT : (n + 1) * NT],
                                start=(k == 0),
                                stop=(k == KCH - 1),
                            )
                        nc.vector.tensor_copy(dst[m][:, n * NT : (n + 1) * NT], ps[:])
            # v projection: out [128(s), 256] over K=E, scatter into v_sb + ones
            for s in range(SCH):
                ps = pp.tile([P, HPC * HD], F32, tag="vps", name="vps")
                for k in range(KCH):
                    nc.tensor.matmul(
                        ps[:],
                        kTc[k][:, s * P : (s + 1) * P],
                        wkv[k][:],
                        start=(k == 0),
                        stop=(k == KCH - 1),
                    )
                vt = v_sb[s]
                for g in range(HPC):
                    nc.vector.tensor_copy(
                        vt[:, g * (HD + 1) : g * (HD + 1) + HD],
                        ps[:, g * HD : (g + 1) * HD],
                    )
                    nc.vector.memset(vt[:, g * (HD + 1) + HD : (g + 1) * (HD + 1)], 1.0)

        # ---- attention (64x128 row-tiled PE mode throughout) ---------------
        with (
            tc.tile_pool(name="pp_sc", bufs=4, space="PSUM") as pp_sc,
            tc.tile_pool(name="pp_av", bufs=4, space="PSUM") as pp_av,
            tc.tile_pool(name="ep", bufs=4) as ep,
            tc.tile_pool(name="np_", bufs=3) as npool,
        ):
            for p in range(2):  # head pairs; global heads 2p (rows 0:64), 2p+1 (64:128)
                for tt in range(T // NT):
                    av = [
                        [pp_av.tile([P, NT], F32, tag="av", name="av") for _ in range(2)]
                        for _ in range(2)
                    ]
                    for s in range(SCH):
                        sc = [pp_sc.tile([P, NT], F32, tag="sc", name="sc") for _ in range(2)]
                        et = [ep.tile([P, NT], BF16, tag="exp", name="exp") for _ in range(2)]
                        for hh in range(2):
                            lo, hi = hh * 64, hh * 64 + 64
                            nc.tensor.matmul(
                                sc[hh][:],
                                kt_sb[p][lo:hi, s * P : (s + 1) * P],
                                qt_sb[p][lo:hi, tt * NT : (tt + 1) * NT],
                                start=True,
                                stop=True,
                                tile_position=(lo, 0),
                            )
                            nc.scalar.activation(et[hh][:], sc[hh][:], EXPF)
                        for hh in range(2):
                            g = 2 * p + hh
                            c0 = g * (HD + 1)
                            for half in range(2):
                                lo, hi = half * 64, half * 64 + 64
                                nc.tensor.matmul(
                                    av[hh][half][0 : HD + 1, :],
                                    v_sb[s][lo:hi, c0 : c0 + HD + 1],
                                    et[hh][lo:hi, :],
                                    start=(s == 0),
                                    stop=(s == SCH - 1),
                                    tile_position=(lo, 0),
                                )
                    for hh in range(2):
                        half0 = npool.tile([P, NT], F32, tag="half0", name="half0")
                        nc.vector.tensor_copy(half0[0 : HD + 1, :], av[hh][0][0 : HD + 1, :])
                        tmp = npool.tile([P, NT], F32, tag="tmp", name="tmp")
                        nc.vector.tensor_add(
                            tmp[0 : HD + 1, :],
                            half0[0 : HD + 1, :],
                            av[hh][1][0 : HD + 1, :],
                        )
                        rec = npool.tile([P, NT], F32, tag="rec", name="rec")
                        nc.vector.reciprocal(rec[0:1, :], tmp[HD : HD + 1, :])
                        nc.gpsimd.partition_broadcast(rec[0:HD, :], rec[0:1, :])
                        nc.vector.tensor_mul(
                            outt_sb[p][hh * HD : (hh + 1) * HD, tt * NT : (tt + 1) * NT],
                            tmp[0:HD, :],
                            rec[0:HD, :],
                        )

        # ---- output projection ---------------------------------------------
        with (
            tc.tile_pool(name="pp_y", bufs=4, space="PSUM") as pp_y,
            tc.tile_pool(name="ysb", bufs=3) as ysb,
        ):
            for m in range(TCH):
                yt = ysb.tile([P, E], F32, tag="y", name="ysb")
                for n in range(E // NT):
                    ps = pp_y.tile([P, NT], F32, tag="yps", name="yps")
                    for k in range(2):
                        nc.tensor.matmul(
                            ps[:],
                            outt_sb[k][:, m * P : (m + 1) * P],
                            wo[k][:, n * NT : (n + 1) * NT],
                            start=(k == 0),
                            stop=(k == 1),
                        )
                    nc.vector.tensor_copy(yt[:, n * NT : (n + 1) * NT], ps[:])
                nc.sync.dma_start(yp.ap()[m * P : (m + 1) * P, :], yt[:])

        # ---- on-device partial-sum + emit bf16 slice -----------------------
        nc.gpsimd.collective_compute(
            "ReduceScatter",
            mybir.AluOpType.add,
            replica_groups=BATCH_GROUPS,
            ins=[yp.ap().opt()],
            outs=[yr.ap().opt()],
        )
        with tc.tile_pool(name="cast", bufs=2) as cast:
            for m in range(TQ // P):
                t32 = cast.tile([P, E], F32, tag="c32", name="c32")
                nc.sync.dma_start(t32[:], yr.ap()[m * P : (m + 1) * P, :])
                t16 = cast.tile([P, E], BF16, tag="c16", name="c16")
                nc.vector.tensor_copy(t16[:], t32[:])
                nc.sync.dma_start(y_d[m * P : (m + 1) * P, :], t16[:])

    if not nc.is_finalized():
        nc.finalize()
    _prog_cache["nc"] = nc
    return nc


def kernel(query, key, value, Wq, bq, Wkv, bkv, Wo, bo):
    query = np.asarray(query, np.float32)
    key = np.asarray(key, np.float32)
    Wq = np.asarray(Wq, np.float32)
    Wkv = np.asarray(Wkv, np.float32)
    Wo = np.asarray(Wo, np.float32)

    bf = ml_dtypes.bfloat16
    # fold the 1/sqrt(HD) score scale into Wq
    Wq_s = (Wq * SCALE).astype(bf)
    Wkv_b = Wkv.astype(bf)
    Wo_b = Wo.astype(bf)

    in_maps = []
    for c in range(8):
        b, hg = divmod(c, 4)
        col = 256 * hg + P * b  # this core's 128-wide weight slice
        in_maps.append(
            {
                "qp": query[b, hg * TQ : (hg + 1) * TQ, :].T.astype(bf),
                "kp": key[b, hg * TQ : (hg + 1) * TQ, :].T.astype(bf),
                "wq": np.ascontiguousarray(Wq_s[:, col : col + P]),
                "wkk": np.ascontiguousarray(Wkv_b[:, col : col + P]),
                "wkv": np.ascontiguousarray(Wkv_b[:, E + col : E + col + P]),
                "wo": np.ascontiguousarray(Wo_b[col : col + P, :]),
            }
        )

    global _last_in_maps
    _last_in_maps = in_maps
    nc = _build_program()
    res = run_bass_kernel_spmd(nc, in_maps, list(range(8)))
    out = np.empty((B, T, E), np.float32)
    for c in range(8):
        b, hg = divmod(c, 4)
        out[b, hg * TQ : (hg + 1) * TQ, :] = np.asarray(
            res.results[c]["y"], np.float32
        )
    out += np.asarray(bo, np.float32)
    return out
